# revision 1
# baseline (speedup 1.0000x reference)
"""Multi-head causal attention (B=4, S=2048, E=1024, H=16, Dh=64) on 8
Trainium2 NeuronCores.

Sharding: data-parallel over the 4 batch elements x tensor-parallel over
heads (2 groups of 8). Core 2b+g handles batch b, heads 8g..8g+7.

v2 design (vs the 493us baseline):
- Startup: inputs are host-swizzled to [P, ktile, free] so every DMA row is
  contiguous, and the first-needed 5MB (wq, xt chunks 0/1 halves) is spread
  over all five engine DMA queues so the first matmul starts ~8us in (was
  34us).
- Stage B softmax is split across two engines: even local heads get the
  exact ACT-engine exp (the causal mask is pre-added on the PE by an
  identity-stationary accumulate matmul), odd local heads get a Schraudolph
  bit-trick exp on the DVE (one fused tensor_scalar mult+add writing int16
  bf16 bits; the mask folds into the add operand). This removes the ACT
  throughput wall that forced the PE down to its mid p-state.
- probs and V are bf16 (AV matmuls run 1 cycle/row at any width, so the
  r=3 diagonal AV shrinks to N=128), scores/projections stay f32r.
- The output projection + bias + store for chunk c are interleaved into
  chunk c+1's attention via a deferred-work backlog, sharing one PSUM bank;
  stores ride the vector/tensor queues so they never queue behind loads.
- Cross-group software pipelining: a group's last AV and its psum
  evacuations are emitted after the next group's first scores, so the PE
  never drains at group boundaries.

The two partial projections per batch are summed on the host (the TP
"all-reduce", done at gather time), which also absorbs the out-transpose:
the kernel emits out^T [E, S].
"""

import json
import math
import sys

for _p in ("/opt/trn_rl_repo",):
    if _p not in sys.path:
        sys.path.insert(0, _p)

import numpy as np

# ---------------------------------------------------------------- constants
B = 4
S = 2048
E = 1024
H = 16
DH = 64
HL = 8  # heads per core
DL = HL * DH  # 512, local head dim
P = 128
NCORES = 8
SCALE = 1.0 / 8.0  # 1/sqrt(DH)
MBIG = 480.0  # additive causal mask magnitude (unscaled-score units)

KT_E = E // P  # 8  k-tiles over embed dim
MT = DL // P  # 4  m-tiles over local head dim (2 heads per m-tile)
SC = S // 512  # 4  512-wide chunks over sequence
SB = S // P  # 16 128-blocks over sequence
KT_D = DL // P  # 4  k-tiles over local head dim (proj contraction)
MT_E = E // P  # 8  m-tiles over embed dim (proj output)
VW = DH + 1  # 65: V columns per head + ones column

# Schraudolph exp in bf16: bits(exp(s*SCALE)) ~ int16(s * A16P + B16F)
A16P = float((2.0**7) / math.log(2.0) * SCALE)
B16F = float(127 * 128 - 7.0)


# ------------------------------------------------- BIR multi-wait splitting
# The walrus build here accepts one sync-wait command per instruction; Tile
# attaches every outstanding dependency to the consumer. Split extras into
# single-wait EventSemaphore instructions just before the consumer (same
# engine => same blocking behavior).
_syncfix_done = [False]


def _install_syncfix():
    if _syncfix_done[0]:
        return
    _syncfix_done[0] = True
    import concourse.bass_utils as bu

    counter = [0]

    def split_multiwait(bir_json):
        d = json.loads(bir_json)
        changed = False
        for fn in d.get("functions", []):
            for bb in fn.get("blocks", []):
                new_insts = []
                for inst in bb.get("instructions", []):
                    si = inst.get("sync_info")
                    waits = (si or {}).get("on_wait") or []
                    if len(waits) > 1:
                        changed = True
                        for w in waits[:-1]:
                            counter[0] += 1
                            new_insts.append(
                                {
                                    "debug": inst.get("debug"),
                                    "engine": inst["engine"],
                                    "ins": [],
                                    "name": f"WSPLIT-{counter[0]}",
                                    "opcode": "EventSemaphore",
                                    "outs": [],
                                    "sync_info": {"on_update": [], "on_wait": [w]},
                                }
                            )
                        si["on_wait"] = [waits[-1]]
                    new_insts.append(inst)
                bb["instructions"] = new_insts
        if not changed:
            return bir_json if isinstance(bir_json, bytes) else bir_json.encode()
        return json.dumps(d).encode()

    orig = bu.compile_bir_kernel

    def patched(bir_json, tmpdir, neff_name="file.neff"):
        return orig(split_multiwait(bir_json), tmpdir, neff_name)

    bu.compile_bir_kernel = patched
    try:
        import concourse.bass2jax as b2j

        if hasattr(b2j, "compile_bir_kernel"):
            b2j.compile_bir_kernel = patched
    except ImportError:
        pass


# ------------------------------------------------------------ kernel build
def build_nc():
    import concourse.bass as bass
    import concourse.tile as tile
    from concourse import mybir

    f32 = mybir.dt.float32
    f32r = mybir.dt.float32r
    bf16 = mybir.dt.bfloat16
    i16 = mybir.dt.int16
    EXP = mybir.ActivationFunctionType.Exp
    IDENT = mybir.ActivationFunctionType.Identity
    MULT = mybir.AluOpType.mult
    ADD = mybir.AluOpType.add

    nc = bass.Bass()

    xt_ext = nc.dram_tensor("xt", [P, KT_E, S], f32r, kind="ExternalInput")
    wq_ext = nc.dram_tensor("wq", [P, KT_E, DL], f32r, kind="ExternalInput")
    wk_ext = nc.dram_tensor("wk", [P, KT_E, DL], f32r, kind="ExternalInput")
    wv_ext = nc.dram_tensor("wv", [P, KT_E, DL], f32r, kind="ExternalInput")
    wo_ext = nc.dram_tensor("wo", [P, KT_D, E], bf16, kind="ExternalInput")
    bo_ext = nc.dram_tensor("bo2", [E], f32, kind="ExternalInput")
    maskr_ext = nc.dram_tensor("maskr", [P, P], bf16, kind="ExternalInput")
    idt_ext = nc.dram_tensor("idt", [P, P], bf16, kind="ExternalInput")
    bmask_ext = nc.dram_tensor("bmask", [P, P], f32, kind="ExternalInput")
    o64_ext = nc.dram_tensor("o64", [1, DH], bf16, kind="ExternalInput")
    out_ext = nc.dram_tensor("outp", [E, S], f32, kind="ExternalOutput")

    bo_r = bo_ext.rearrange("(m p) -> p m", p=P)

    with tile.TileContext(nc) as tc:
        with tc.tile_pool(name="persist", bufs=1) as pers:
            # ---- persistent SBUF tensors
            qt = [pers.tile([P, S], bf16, tag=f"qt{m}", name=f"qt{m}") for m in range(MT)]
            kt = [pers.tile([P, S], bf16, tag=f"kt{m}", name=f"kt{m}") for m in range(MT)]
            v_sb = pers.tile([P, SB, HL * VW], bf16, tag="v")
            bo_sb = pers.tile([P, MT_E], f32, tag="bo")
            maskr_sb = pers.tile([P, P], bf16, tag="maskr")
            idt_sb = pers.tile([P, P], bf16, tag="idt")
            bmask_sb = pers.tile([P, P], f32, tag="bmask")
            o64_sb = pers.tile([1, DH], bf16, tag="o64")

            # ---- stage A: QT/KT (transposed) and V (natural) projections.
            with (
                tc.tile_pool(name="wqkv", bufs=1) as wpool,
                tc.tile_pool(name="xt", bufs=4) as xpool,
                tc.tile_pool(name="ps_a", bufs=6, space="PSUM") as ps_a,
            ):
                # Weights and xt chunks are split into k-half TILES so a
                # matmul's dependency covers only the half it reads — the
                # first k0-3 matmuls start once 1MB (7.4us) has landed.
                KH = KT_E // 2  # 4: half the k-tiles
                wq_sb = [
                    wpool.tile([P, KH, DL], f32r, tag=f"wq{i}", name=f"wq{i}")
                    for i in range(2)
                ]
                wk_sb = [
                    wpool.tile([P, KH, DL], f32r, tag=f"wk{i}", name=f"wk{i}")
                    for i in range(2)
                ]
                wv_sb = [
                    wpool.tile([P, KH, DL], f32r, tag=f"wv{i}", name=f"wv{i}")
                    for i in range(2)
                ]
                xt_tiles = {}

                # 3 DMA queues (sync / scalar / gpsimd) share ~360GB/s of
                # HBM. Issue 0.5MB pieces round-robin in global need order
                # so the bytes the first matmuls block on arrive first.
                rr = [nc.sync, nc.scalar, nc.gpsimd]
                qi = [0]

                def piece(dst, src):
                    rr[qi[0] % 3].dma_start(dst, src)
                    qi[0] += 1

                def mk_xc(c):
                    xc = [
                        xpool.tile(
                            [P, KH, 512], f32r, tag=f"xt{i}", name=f"xt{c}_{i}"
                        )
                        for i in range(2)
                    ]
                    xt_tiles[c] = xc
                    return xc

                def xc_piece(c, half, kh):
                    lo = 2 * kh
                    piece(
                        xt_tiles[c][half][:, lo : lo + 2, :],
                        xt_ext[
                            :, KH * half + lo : KH * half + lo + 2,
                            512 * c : 512 * (c + 1),
                        ],
                    )

                def w_piece(w_sb, w_ext, half, kh):
                    lo = 2 * kh
                    piece(
                        w_sb[half][:, lo : lo + 2, :],
                        w_ext[:, KH * half + lo : KH * half + lo + 2, :],
                    )

                for c in range(SC):
                    mk_xc(c)
                for kh in range(2):
                    w_piece(wq_sb, wq_ext, 0, kh)
                    xc_piece(0, 0, kh)
                    xc_piece(1, 0, kh)
                for kh in range(2):
                    w_piece(wq_sb, wq_ext, 1, kh)
                    xc_piece(0, 1, kh)
                    xc_piece(1, 1, kh)
                for kh in range(2):
                    w_piece(wk_sb, wk_ext, 0, kh)
                    w_piece(wk_sb, wk_ext, 1, kh)
                for kh in range(2):
                    w_piece(wv_sb, wv_ext, 0, kh)
                    w_piece(wv_sb, wv_ext, 1, kh)
                for kh in range(2):
                    xc_piece(2, 0, kh)
                    xc_piece(2, 1, kh)
                for kh in range(2):
                    xc_piece(3, 0, kh)
                    xc_piece(3, 1, kh)
                # small consts
                nc.gpsimd.dma_start(bo_sb[:], bo_r)
                nc.gpsimd.dma_start(maskr_sb[:], maskr_ext[:, :])
                nc.gpsimd.dma_start(idt_sb[:], idt_ext[:, :])
                nc.gpsimd.dma_start(bmask_sb[:], bmask_ext[:, :])
                nc.gpsimd.dma_start(o64_sb[:], o64_ext[:, :])
                ones_col = v_sb[:].rearrange("p sb (h c) -> p sb h c", c=VW)[
                    :, :, :, DH : DH + 1
                ]
                nc.gpsimd.memset(ones_col, 1.0)

                for cp in range(SC // 2):
                    cs = (2 * cp, 2 * cp + 1)
                    xt_sb = {c: xt_tiles[c] for c in cs}

                    # QK^T: psum[m] = sum_k W[k, m-block].T @ XT[k, chunk]
                    for w_sb, dst in ((wq_sb, qt), (wk_sb, kt)):
                        for m in range(MT):
                            pss = {
                                c: ps_a.tile(
                                    [P, 512], f32, tag="ps_a", name=f"psa{m}_{c}"
                                )
                                for c in cs
                            }
                            for k in range(KT_E):
                                kh, ko = divmod(k, KH)
                                for c in cs:
                                    nc.tensor.matmul(
                                        pss[c][:],
                                        w_sb[kh][:, ko, P * m : P * (m + 1)],
                                        xt_sb[c][kh][:, ko, :],
                                        start=(k == 0),
                                        stop=(k == KT_E - 1),
                                    )
                            for c in cs:
                                # psum evac on ACT (idle in stage A)
                                nc.scalar.copy(
                                    dst[m][:, 512 * c : 512 * (c + 1)], pss[c][:]
                                )

                    # V natural: psum[sb] = sum_k XT[k, sblock].T @ WV[k, :]
                    for c in cs:
                        for s in range(4):
                            sb = 4 * c + s
                            ps = ps_a.tile(
                                [P, 512], f32, tag="ps_a", name=f"psv{sb}"
                            )
                            for k in range(KT_E):
                                kh, ko = divmod(k, KH)
                                nc.tensor.matmul(
                                    ps[:],
                                    xt_sb[c][kh][:, ko, P * s : P * (s + 1)],
                                    wv_sb[kh][:, ko, :],
                                    start=(k == 0),
                                    stop=(k == KT_E - 1),
                                )
                            # one strided bf16 copy for all 8 heads
                            nc.vector.tensor_copy(
                                v_sb[:, sb, :].rearrange(
                                    "p (h c) -> p h c", c=VW
                                )[:, :, 0:DH],
                                ps[:].rearrange("p (h c) -> p h c", c=DH),
                            )

            # ---- stages B+C interleaved
            with tc.tile_pool(name="late", bufs=1) as late:
                ct = [
                    late.tile([P, KT_D, 512], bf16, tag=f"ct{i}", name=f"ct{i}")
                    for i in range(SC)
                ]
                wo_sb = late.tile([P, KT_D, E], bf16, tag="wo")
                nc.sync.dma_start(wo_sb[:, 0:2, :], wo_ext[:, 0:2, :])
                nc.gpsimd.dma_start(wo_sb[:, 2:4, :], wo_ext[:, 2:4, :])
                _stage_bc(
                    nc, tc, qt, kt, v_sb, ct, wo_sb, bo_sb, maskr_sb, idt_sb,
                    bmask_sb, o64_sb, out_ext, EXP, IDENT, MULT, ADD,
                    f32, f32r, bf16, i16,
                )

    return nc


def _stage_bc(
    nc, tc, qt, kt, v_sb, ct, wo_sb, bo_sb, maskr_sb, idt_sb, bmask_sb,
    o64_sb, out_ext, EXP, IDENT, MULT, ADD, f32, f32r, bf16, i16,
):
    groups = [(c, t) for c in range(SC) for t in range(MT)]

    def sch_tile(h, j, c):
        # odd local heads go to the DVE Schraudolph exp, except the j%4==0
        # off-diagonal share which returns to ACT to balance engine load
        if h % 2 == 0:
            return False
        diag = j >= 4 * c
        return diag or (j % 4 != 0)

    with (
        tc.tile_pool(name="probs", bufs=8) as ppool,
        tc.tile_pool(name="fin", bufs=2) as fpool,
        tc.tile_pool(name="ostage", bufs=4) as opool,
        tc.tile_pool(name="ps_s", bufs=3, space="PSUM") as ps_s,
        tc.tile_pool(name="ps_ctx", bufs=2, space="PSUM") as ps_ctx,
    ):
        backlog = []
        store_eng = [0]

        def emit_scores_pair(c, t, jp, hh):
            """Scores + exp for kv blocks (2jp, 2jp+1) of local head hh, in
            one [P,1024] two-bank psum tile and ONE exp per engine pass."""
            hp = DH * (hh % 2)
            q_lo = 512 * c
            use_sch = sch_pair(hh, jp, c)
            pr = ppool.tile(
                [P, 1024], bf16, tag="probs", name=f"pr_{c}_{hh}_{jp}"
            )
            ps = ps_s.tile([P, 1024], f32, tag="ps_s")
            lo0 = None
            for half in range(2):
                j = 2 * jp + half
                r = j - 4 * c
                slo = P * r if r in (1, 2) else 0
                if lo0 is None:
                    lo0 = P * r if r > 0 else 0
                diag = r >= 0
                co = 512 * half
                nc.tensor.matmul(
                    ps[:, co + slo : co + 512],
                    kt[t][hp : hp + DH, P * j : P * (j + 1)],
                    qt[t][hp : hp + DH, q_lo + slo : q_lo + 512],
                    start=True,
                    stop=not (diag and not use_sch),
                )
                if diag and not use_sch:
                    # fold the causal mask in on the PE: psum += I.T @ maskr
                    nc.tensor.matmul(
                        ps[:, co + P * r : co + P * (r + 1)],
                        idt_sb[:],
                        maskr_sb[:],
                        start=False,
                        stop=True,
                    )
            if use_sch:
                for half in range(2):
                    j = 2 * jp + half
                    r = j - 4 * c
                    co = 512 * half
                    if r >= 0:
                        nc.vector.scalar_tensor_tensor(
                            pr[:, co + P * r : co + P * (r + 1)].bitcast(i16),
                            ps[:, co + P * r : co + P * (r + 1)],
                            A16P,
                            bmask_sb[:],
                            MULT,
                            ADD,
                        )
                        if r < 3:
                            nc.vector.tensor_scalar(
                                pr[:, co + P * (r + 1) : co + 512].bitcast(i16),
                                ps[:, co + P * (r + 1) : co + 512],
                                A16P,
                                B16F,
                                MULT,
                                ADD,
                            )
                    elif half == 0:
                        # off-diag pair: one fused pass over both halves
                        nc.vector.tensor_scalar(
                            pr[:, 0:1024].bitcast(i16), ps[:], A16P, B16F,
                            MULT, ADD,
                        )
                        break
            else:
                # unwritten psum regions read as 0 -> exp(0)=1, never read
                nc.scalar.activation(
                    pr[:, lo0:1024], ps[:, lo0:1024], EXP, scale=SCALE
                )
            return pr

        def sch_pair(hh, jp, c):
            # odd local heads -> DVE Schraudolph exp, except a share of
            # off-diagonal pairs that returns to ACT for engine balance
            if hh % 2 == 0:
                return False
            diag = 2 * jp + 1 >= 4 * c
            return diag or (jp % 2 != 0)

        def emit_av(st, j, last):
            c, t = st["c"], st["t"]
            r = j - 4 * c
            lo = P * r if r > 0 else 0
            jp, half = divmod(j, 2)
            co = 512 * half
            for i, hh in enumerate((2 * t, 2 * t + 1)):
                nc.tensor.matmul(
                    st["ctx"][i][:, lo:512],
                    v_sb[:, j, VW * hh : VW * (hh + 1)],
                    st["probs"][(i, jp)][:, co + lo : co + 512],
                    start=(j == 0),
                    stop=last,
                )

        def emit_group_close(st):
            # h0 evacuations on ACT, h1 on DVE (engine balance); h0 first so
            # its ctx bank frees earliest (ctx ring is 3 deep)
            c, t = st["c"], st["t"]
            sums_sb = st["sums"]
            for i, hh in enumerate((2 * t, 2 * t + 1)):
                cp = st["ctx"][i]
                eng_copy = nc.scalar.copy if i == 0 else nc.vector.tensor_copy
                eng_copy(
                    ct[c][DH * (hh % 2) : DH * (hh % 2) + DH, t, :], cp[0:DH, :]
                )
                eng_copy(
                    sums_sb[
                        32 * (hh % 4) : 32 * (hh % 4) + 1,
                        512 * (hh // 4) : 512 * (hh // 4) + 512,
                    ],
                    cp[DH : DH + 1, :],
                )

        def emit_half_close(c, sums_sb, half):
            # reciprocal over one 512-wide sums slot (heads 4*half..4*half+3,
            # i.e. groups t = 2*half, 2*half+1), then stage the bf16 rows the
            # bc broadcast matmuls consume; bc work is deferred via backlog
            recs = fpool.tile(
                [P, 512], f32, tag="recs", name=f"recs{c}_{half}", bufs=2
            )
            nc.vector.reciprocal(recs[:], sums_sb[:, 512 * half : 512 * half + 512])
            for hh in range(4 * half, 4 * half + 4):
                # bufs=8: all live recr tiles of a chunk at once — a smaller
                # ring would block the ACT queue on a bc matmul the PE only
                # reaches later (deadlock via in-order engine queues)
                recr = fpool.tile(
                    [1, 512], bf16, tag="recr", name=f"recr{c}_{hh}", bufs=8
                )
                nc.scalar.copy(recr[:], recs[32 * (hh % 4) : 32 * (hh % 4) + 1, :])
                backlog.append(("bc", c, hh, recr))
            if half == 1:
                for m in range(MT_E):
                    backlog.append(("proj", c, m, None))

        def emit_backlog_item(item, pool=None):
            kind, c, idx, recr = item
            # bc/proj psums ride the ps_s ring (no spare banks for a
            # dedicated pool)
            tag, shape = "ps_s", [P, 1024]
            pool = ps_s
            if kind == "bc":
                hh = idx
                bc = pool.tile(shape, f32, tag=tag, name=f"bc{c}_{hh}")
                nc.tensor.matmul(
                    bc[0:DH, 0:512], o64_sb[:], recr[:], start=True, stop=True
                )
                hp = DH * (hh % 2)
                nc.vector.tensor_mul(
                    ct[c][hp : hp + DH, hh // 2, :],
                    ct[c][hp : hp + DH, hh // 2, :],
                    bc[0:DH, 0:512],
                )
            else:
                m = idx
                pp = pool.tile(shape, f32, tag=tag, name=f"pp{c}_{m}")
                for k in range(KT_D):
                    nc.tensor.matmul(
                        pp[:, 0:512],
                        wo_sb[:, k, P * m : P * (m + 1)],
                        ct[c][:, k, :],
                        start=(k == 0),
                        stop=(k == KT_D - 1),
                    )
                ot = opool.tile([P, 512], f32, tag="ostage", name=f"ot{c}_{m}")
                if m % 2 == 0:
                    nc.scalar.activation(
                        ot[:], pp[:, 0:512], IDENT, bias=bo_sb[:, m : m + 1]
                    )
                else:
                    nc.vector.tensor_scalar(
                        ot[:], pp[:, 0:512], bo_sb[:, m : m + 1], None, ADD
                    )
                eng = nc.sync if store_eng[0] % 2 == 0 else nc.gpsimd
                store_eng[0] += 1
                eng.dma_start(
                    out_ext[P * m : P * (m + 1), 512 * c : 512 * (c + 1)], ot[:]
                )

        def close_prev(prev, stage):
            # carried work from the previous group, spread over the first two
            # pair-steps of the next group
            if prev is None:
                return
            nj = prev["nj"]
            if stage == 0:
                emit_av(prev, nj - 4, last=False)
                emit_av(prev, nj - 3, last=False)
            else:
                emit_av(prev, nj - 2, last=False)
                emit_av(prev, nj - 1, last=True)
                emit_group_close(prev)
                if prev["t"] == 1:
                    emit_half_close(prev["c"], prev["sums"], 0)
                elif prev["t"] == 3:
                    emit_half_close(prev["c"], prev["sums"], 1)

        prev = None
        for c, t in groups:
            nj = 4 * c + 4
            st = {
                "c": c,
                "t": t,
                "nj": nj,
                "probs": {},
                "ctx": [
                    ps_ctx.tile([VW, 512], f32, tag="ps_ctx", name=f"ctx{c}_{t}_{i}")
                    for i in range(2)
                ],
                "sums": (
                    fpool.tile([P, 1024], f32, tag="sums", name=f"sums{c}")
                    if t == 0
                    else prev["sums"]
                ),
            }
            for jp in range(nj // 2):
                for i, hh in enumerate((2 * t, 2 * t + 1)):
                    st["probs"][(i, jp)] = emit_scores_pair(c, t, jp, hh)
                if jp <= 1:
                    close_prev(prev, jp)
                else:
                    emit_av(st, 2 * jp - 4, last=False)
                    emit_av(st, 2 * jp - 3, last=False)
                    if jp >= 2 and backlog:
                        emit_backlog_item(backlog.pop(0))
            prev = st

        # tail: last group's AVs + close, last chunk's normalization + proj.
        # Tail proj psums alternate with the now-idle ps_s ring so the eight
        # projections pipeline instead of serializing on the single misc bank.
        close_prev(prev, 0)
        close_prev(prev, 1)
        tail_items = list(backlog)
        backlog.clear()
        for item in tail_items:
            emit_backlog_item(item)


# ------------------------------------------------------------ PJRT runner
class _Runner:
    """Compile once, run many: mirrors bass2jax.run_bass_via_pjrt with a
    cached jitted executable."""

    def __init__(self, nc):
        import jax
        import jax.numpy  # noqa: F401
        from jax.sharding import Mesh, PartitionSpec
        from jax.experimental.shard_map import shard_map
        import concourse.bass2jax as b2j
        from concourse import mybir

        b2j.install_neuronx_cc_hook()
        self.jax = jax
        partition_name = (
            nc.partition_id_tensor.name if nc.partition_id_tensor else None
        )
        in_names = []
        out_names = []
        out_avals = []
        self.zero_shapes = []
        for alloc in nc.m.functions[0].allocations:
            if not isinstance(alloc, mybir.MemoryLocationSet):
                continue
            name = alloc.memorylocations[0].name
            if alloc.kind == "ExternalInput":
                if name == partition_name:
                    continue
                in_names.append(name)
            elif alloc.kind == "ExternalOutput":
                shape = tuple(alloc.tensor_shape)
                dtype = mybir.dt.np(alloc.dtype)
                out_names.append(name)
                out_avals.append(jax.core.ShapedArray(shape, dtype))
                self.zero_shapes.append((shape, dtype))
        self.in_names = in_names
        self.out_names = out_names
        self.out_avals = out_avals
        n_params = len(in_names)
        n_outs = len(out_avals)
        all_in = list(in_names) + list(out_names)
        if partition_name is not None:
            all_in.append(partition_name)

        def _body(*args):
            operands = list(args)
            if partition_name is not None:
                operands.append(b2j.partition_id_tensor())
            outs = b2j._bass_exec_p.bind(
                *operands,
                out_avals=tuple(out_avals),
                in_names=tuple(all_in),
                out_names=tuple(out_names),
                lowering_input_output_aliases=(),
                sim_require_finite=True,
                sim_require_nnan=True,
                nc=nc,
            )
            return tuple(outs)

        devices = jax.devices()[:NCORES]
        assert len(devices) == NCORES, f"need {NCORES} cores, got {len(devices)}"
        self.mesh = Mesh(np.asarray(devices), ("core",))
        in_specs = (PartitionSpec("core"),) * (n_params + n_outs)
        out_specs = (PartitionSpec("core"),) * n_outs
        self.fn = jax.jit(
            shard_map(
                _body,
                mesh=self.mesh,
                in_specs=in_specs,
                out_specs=out_specs,
                check_rep=False,
            ),
            donate_argnums=tuple(range(n_params, n_params + n_outs)),
            keep_unused=True,
        )

    def run(self, in_maps):
        concat_in = [
            np.concatenate([np.asarray(m[name]) for m in in_maps], axis=0)
            for name in self.in_names
        ]
        zeros = [
            np.zeros((NCORES * s[0], *s[1:]), dt) for s, dt in self.zero_shapes
        ]
        outs = self.fn(*concat_in, *zeros)
        return [
            {
                name: np.asarray(outs[i]).reshape(
                    NCORES, *self.out_avals[i].shape
                )[c]
                for i, name in enumerate(self.out_names)
            }
            for c in range(NCORES)
        ]


_cache = {}


def _get_runner():
    if "runner" not in _cache:
        _install_syncfix()
        _cache["runner"] = _Runner(build_nc())
    return _cache["runner"]


def make_in_maps(X, Wq, Wk, Wv, Wo, bo):
    import ml_dtypes

    X = np.asarray(X, dtype=np.float32)
    Wq = np.asarray(Wq, dtype=np.float32)
    Wk = np.asarray(Wk, dtype=np.float32)
    Wv = np.asarray(Wv, dtype=np.float32)
    Wo = np.asarray(Wo, dtype=np.float32)
    bo = np.asarray(bo, dtype=np.float32)

    kv = np.arange(P)[:, None]
    qq = np.arange(P)[None, :]
    tri = kv > qq
    maskr = np.where(tri, np.float32(-MBIG), np.float32(0.0)).astype(
        ml_dtypes.bfloat16
    )
    idt = np.eye(P, dtype=ml_dtypes.bfloat16)
    bmask = np.where(
        tri, np.float32(B16F - MBIG * A16P), np.float32(B16F)
    ).astype(np.float32)

    def swiz_w(w):  # [E, DL] -> [P, KT_E, DL]
        return np.ascontiguousarray(
            w.reshape(KT_E, P, DL).transpose(1, 0, 2)
        )

    in_maps = []
    for core in range(NCORES):
        b, g = divmod(core, 2)
        h0 = HL * g
        xt = np.ascontiguousarray(
            X[b].T.reshape(KT_E, P, S).transpose(1, 0, 2)
        )
        in_maps.append(
            {
                "xt": xt,
                "wq": swiz_w(Wq[h0 : h0 + HL].transpose(1, 0, 2).reshape(E, DL)),
                "wk": swiz_w(Wk[h0 : h0 + HL].transpose(1, 0, 2).reshape(E, DL)),
                "wv": swiz_w(Wv[h0 : h0 + HL].transpose(1, 0, 2).reshape(E, DL)),
                "wo": np.ascontiguousarray(
                    Wo[:, DL * g : DL * (g + 1)].T.reshape(KT_D, P, E)
                    .transpose(1, 0, 2)
                ).astype(ml_dtypes.bfloat16),
                "bo2": bo if g == 0 else np.zeros_like(bo),
                "maskr": maskr,
                "idt": idt,
                "bmask": bmask,
                "o64": np.ones((1, DH), dtype=ml_dtypes.bfloat16),
            }
        )
    return in_maps


def assemble(results):
    out = np.empty((B, S, E), dtype=np.float32)
    for b in range(B):
        acc = results[2 * b]["outp"] + results[2 * b + 1]["outp"]
        out[b] = acc.T
    return out


def kernel(X, Wq, Wk, Wv, Wo, bo):
    runner = _get_runner()
    in_maps = make_in_maps(X, Wq, Wk, Wv, Wo, bo)
    results = runner.run(in_maps)
    return assemble(results)

